# revision 15
# baseline (speedup 1.0000x reference)
"""Multi-head attention (B=2, N=4096, C=768, H=12, D=64) on 8 TRN2 NeuronCores.

Sharding: tensor-parallel over (batch, head). B*H = 24 pairs -> 3 per core.
Cores 0-3 handle batch 0, cores 4-7 batch 1 (3 consecutive heads each).
Each core computes the QKV projection, attention, and a partial output
projection for its heads, returning a partial y^T [768, 4096]. The host
sums the 4 partials per batch, transposes, and adds the bias.

Key performance structure (v2, vs the paced baseline):

1. MM1 (S^T = K_chunk @ q^T) has contraction D=64 — half the PE array.
   Both 64-row halves of the array run CONCURRENTLY via tile_position row
   tiling (auto-derived from base partitions): heads h0/h1 are packed on
   partitions 0:64 / 64:128 of qt01/kt01, so emitting their chunk-matmuls
   back-to-back makes them co-execute (2x MM1 throughput). Head h2 is
   paired with itself across even/odd k-chunks, using q2/k2 duplicated on
   both partition halves (qt2d/kt2d).
2. The softmax exp (50M elements/core) saturates ScalarE (~327us alone),
   so it is split: ScalarE does true exp on the leading column span of
   each PSUM score group; the DVE computes a Schraudolph bit-trick exp
   (int32(s*A+B) reinterpreted as fp32) on the rest in a single
   tensor_scalar pass. P is stored fp32; MM2 runs with float32r operands
   (same PE rate as bf16 at N=512).
3. MM2 keeps the ones-column trick (v_aug M=65) for softmax denominators.
4. Output projection: wp01 part is K=128; the wp2 (K=64) parts of two
   adjacent output chunks are row-tiled concurrently using wp2/ot2
   duplicated on both partition halves.

Per-j schedule, heads h0+h1 (loop1): groups of 3 k-chunks; per group the
PE does [3 MM1-pair slots][6 deferred MM2 of prev group] while ScalarE+DVE
exp the previous S banks; S PSUM single-buffered [128,3072] (6 banks) +
o_t0/o_t1 (2 banks) = 16KB exactly. Loop2 does h2 (groups of 4 chunks via
even/odd pairing) plus the deferred output projection of the previous j.
"""

import os

import ml_dtypes
import numpy as np

import concourse.bass as bass
import concourse.mybir as mybir
import concourse.tile as tile
from concourse import bacc
from concourse.bass_utils import run_bass_kernel_spmd

F32 = mybir.dt.float32
F32R = mybir.dt.float32r
I32 = mybir.dt.int32
BF16 = mybir.dt.bfloat16

DIM = 768
NUM_HEADS = 12
HEAD_DIM = 64
SCALE = HEAD_DIM ** -0.5
B = 2
N_FULL = 4096
N_CORES = 8
HEADS_PER_CORE = 3
CC = DIM // 128  # 6 contraction chunks

# Schraudolph fast-exp constants, bf16 variant:
# exp(s) ~= bitcast_bf16(int16(s*EXPA + EXPB)). bf16 is the top half of
# fp32, so the exponent field sits at bit 7. EXPB centers the sawtooth
# error (~+-3.4% max) and includes rounding compensation for the fp->int
# truncation (values are always positive here).
EXPA = 184.6650292  # 2^7 / ln(2)
EXPB = 16250.5      # 127*2^7 - centering

# Column split of each exp group between ScalarE (true exp, [0:SC)) and
# DVE (bit-trick, [SC:end)). Tuned so each engine stays under the PE time.
SC1_COLS = 1792  # of 3072 (loop1 full groups)
SC2_COLS = 1280  # of 2048 (loop2 groups)


def build_nc(n=N_FULL, fast_mm=True, pace=True):
    """Build the per-core Bass program. Same program runs SPMD on all
    cores; per-core inputs differ (x^T batch + per-head weight slices)."""
    nj = n // 512      # q slices
    nk = n // 128      # k chunks
    md = BF16 if fast_mm else F32R

    nc = bacc.Bacc("TRN2", target_bir_lowering=False, debug=False)

    xt_d = nc.dram_tensor("xt", [DIM, n], md, kind="ExternalInput")
    wqk_d = nc.dram_tensor("wqk", [DIM, 384], md, kind="ExternalInput")
    wv_d = nc.dram_tensor("wv", [DIM, 192], md, kind="ExternalInput")
    wp_d = nc.dram_tensor("wp", [192, DIM], md, kind="ExternalInput")
    yt_d = nc.dram_tensor("yt", [DIM, n], F32, kind="ExternalOutput")

    # loop1 (h0+h1) k-chunk groups of 3; loop2 (h2) groups of 4.
    groups1 = [3] * (nk // 3)
    if nk % 3:
        groups1.append(nk % 3)
    ng2 = nk // 4

    lp = nc.allow_low_precision(
        reason="bf16 matmul operands; fp32 PSUM accumulation; fast-exp "
               "bit trick on part of the softmax within tolerance")
    with lp, tile.TileContext(nc) as tc:
        consts = tc.alloc_tile_pool(name="consts", bufs=1)
        persist = tc.alloc_tile_pool(name="persist", bufs=1)

        wqk_sb = consts.tile([128, CC, 384], md, tag="wqk")
        wqk_r = wqk_d[:, :].rearrange("(a p) m -> p a m", p=128)
        nc.sync.dma_start(out=wqk_sb[:, 0:2, :], in_=wqk_r[:, 0:2, :])
        nc.sync.dma_start(out=wqk_sb[:, 2:CC, :], in_=wqk_r[:, 2:CC, :])
        wv_sb = consts.tile([128, CC, 192], md, tag="wv")
        nc.sync.dma_start(out=wv_sb, in_=wv_d[:, :].rearrange("(a p) m -> p a m", p=128))
        wp01_sb = consts.tile([128, DIM], md, tag="wp01")
        nc.gpsimd.dma_start(out=wp01_sb, in_=wp_d[0:128, :])
        # wp2 duplicated on both partition halves for row-tiled outproj
        wp2d_sb = consts.tile([128, DIM], md, tag="wp2d")
        nc.gpsimd.dma_start(out=wp2d_sb[0:64, :], in_=wp_d[128:192, :])
        nc.gpsimd.dma_start(out=wp2d_sb[64:128, :], in_=wp_d[128:192, :])

        # Persistent activations.
        qt01 = persist.tile([128, n], md, tag="qt01")  # parts 0:64 h0, 64:128 h1
        kt01 = persist.tile([128, n], md, tag="kt01")
        qt2d = persist.tile([128, n], md, tag="qt2d")  # q2 duplicated both halves
        kt2d = persist.tile([128, n], md, tag="kt2d")  # k2 duplicated both halves
        v_aug = [persist.tile([128, nk, 65], md, tag=f"vaug{h}", name=f"vaug{h}")
                 for h in range(HEADS_PER_CORE)]
        for h in range(HEADS_PER_CORE):
            ones_col = v_aug[h][:, :, 64:65]
            if md == F32R:
                ones_col = ones_col.bitcast(F32)
            nc.vector.memset(ones_col, 1.0)
        # Normalized per-head outputs, persist until the output projection.
        ot01s = [persist.tile([128, 512], md, tag=f"ot01_{j}", name=f"ot01_{j}")
                 for j in range(nj)]
        ot2ds = [persist.tile([128, 512], md, tag=f"ot2d_{j}", name=f"ot2d_{j}")
                 for j in range(nj)]

        # Preload the exp table set while ScalarE is otherwise idle so the
        # ~2.7us ACT_TABLE_LOAD doesn't gate the first attention group.
        actwarm = consts.tile([1, 2], F32, tag="actwarm")
        nc.vector.memset(actwarm, 0.0)
        nc.scalar.activation(out=actwarm[0:1, 1:2], in_=actwarm[0:1, 0:1],
                             func=mybir.ActivationFunctionType.Exp)

        # ---- Phase A: QKV projections from pre-transposed x ----
        with (
            tc.tile_pool(name="xtj", bufs=2) as xtj_p,
            tc.tile_pool(name="qk_ps", bufs=1, space="PSUM") as qk_ps,
            tc.tile_pool(name="v_ps", bufs=1, space="PSUM") as v_ps,
        ):
            for j in range(nj):
                jsl = bass.ts(j, 512)
                xtj = xtj_p.tile([128, CC, 512], md, tag="xtj")
                xt_r = xt_d[:, jsl].rearrange("(a p) m -> p a m", p=128)
                if j == 0:
                    nc.sync.dma_start(out=xtj[:, 0:2, :], in_=xt_r[:, 0:2, :])
                    nc.sync.dma_start(out=xtj[:, 2:CC, :], in_=xt_r[:, 2:CC, :])
                else:
                    nc.sync.dma_start(out=xtj, in_=xt_r)
                # q/k projections: packs [q0|q1], [k0|k1], [q2|k2]
                for pi, colbase in enumerate((0, 128, 256)):
                    ps = qk_ps.tile([128, 512], F32, tag="qk")
                    for cc in range(CC):
                        nc.tensor.matmul(
                            ps,
                            wqk_sb[:, cc, colbase:colbase + 128],
                            xtj[:, cc, :],
                            start=(cc == 0), stop=(cc == CC - 1),
                        )
                    if pi == 0:
                        nc.vector.tensor_copy(out=qt01[:, j * 512:j * 512 + 256], in_=ps[:, 0:256])
                        nc.vector.tensor_copy(out=qt01[:, j * 512 + 256:j * 512 + 512], in_=ps[:, 256:512])
                    elif pi == 1:
                        nc.vector.tensor_copy(out=kt01[:, j * 512:j * 512 + 256], in_=ps[:, 0:256])
                        nc.vector.tensor_copy(out=kt01[:, j * 512 + 256:j * 512 + 512], in_=ps[:, 256:512])
                    else:
                        # q2 -> lo directly, k2 -> hi directly; then DMA
                        # partition-shifts create the duplicated halves.
                        nc.vector.tensor_copy(out=qt2d[0:64, jsl], in_=ps[0:64, :])
                        nc.vector.tensor_copy(out=kt2d[64:128, jsl], in_=ps[64:128, :])
                        nc.sync.dma_start(out=qt2d[64:128, jsl], in_=qt2d[0:64, jsl])
                        nc.sync.dma_start(out=kt2d[0:64, jsl], in_=kt2d[64:128, jsl])

                # v projection (natural orientation), 3 heads
                for rc in range(4):
                    ps = v_ps.tile([128, 192], F32, tag="v")
                    for cc in range(CC):
                        nc.tensor.matmul(
                            ps,
                            xtj[:, cc, bass.ts(rc, 128)],
                            wv_sb[:, cc, :],
                            start=(cc == 0), stop=(cc == CC - 1),
                        )
                    kc = j * 4 + rc
                    for h in range(HEADS_PER_CORE):
                        nc.vector.tensor_copy(
                            out=v_aug[h][:, kc, 0:64], in_=ps[:, bass.ts(h, 64)]
                        )

        exp_dve = os.environ.get("EXP_DVE", "1") == "1"

        def emit_exp(s_t, pts_t, ptv_t, ncols):
            """exp of s_t[:, 0:ncols] (PSUM). ScalarE true-exps the first
            half into pts_t (bf16); DVE bit-trick-exps the second half into
            ptv_t (int16 tile holding bf16 bit patterns). Separate output
            tiles keep the two writes dependency-free so they overlap."""
            half = ncols // 2
            nc.scalar.activation(
                out=pts_t[:, 0:half], in_=s_t[:, 0:half],
                func=mybir.ActivationFunctionType.Exp,
            )
            if exp_dve:
                nc.vector.tensor_scalar(
                    out=ptv_t[:, 0:ncols - half],
                    in0=s_t[:, half:ncols],
                    scalar1=float(EXPA), scalar2=float(EXPB),
                    op0=mybir.AluOpType.mult, op1=mybir.AluOpType.add,
                )
            else:
                nc.scalar.activation(
                    out=ptv_t[:, 0:ncols - half].bitcast(md),
                    in_=s_t[:, half:ncols],
                    func=mybir.ActivationFunctionType.Exp,
                )

        def p_slice(pts_t, ptv_t, half, c):
            """MM2 rhs for the 512-col chunk at interleaved column c."""
            if c < half:
                return pts_t[:, c:c + 512]
            return ptv_t[:, c - half:c - half + 512].bitcast(md)

        def normalize(o_t, dst64, obp, rsbp, dscr_p):
            """Drain o_t [65,512] PSUM, divide rows 0:64 by row 64, write
            the normalized bf16 result to dst64 (a [64,512]-shaped AP on
            partitions 0:64). Returns nothing; all off the PE."""
            ob = obp.tile([128, 512], F32, tag="ob")
            nc.vector.tensor_copy(out=ob[0:65, :], in_=o_t[0:65, :])
            scr = dscr_p.tile([512], F32, tag="scr")
            nc.sync.dma_start(out=scr, in_=ob[64:65, :])
            r4 = rsbp.tile([128, 4], F32, tag="r4")
            nc.sync.dma_start(out=r4, in_=scr.rearrange("(p f) -> p f", p=128))
            r4i = rsbp.tile([128, 4], F32, tag="r4i")
            nc.vector.reciprocal(out=r4i, in_=r4)
            scr2 = dscr_p.tile([512], F32, tag="scr2")
            nc.sync.dma_start(out=scr2, in_=r4i)
            bcs = rsbp.tile([64, 512], F32, tag="bcs")
            scr_b = bass.AP(tensor=scr2.tensor, offset=scr2.offset,
                            ap=[[0, 64]] + list(scr2.ap))
            nc.sync.dma_start(out=bcs, in_=scr_b)
            nc.vector.tensor_mul(dst64, ob[0:64, :], bcs)

        # ---- Loop 1: heads h0+h1 attention (row-tiled MM1 pairs) ----
        # S layout interleaves heads by slot: slot t writes h0 -> col 2t*512,
        # h1 -> (2t+1)*512, so ScalarE's half of the group is complete early.
        with (
            tc.tile_pool(name="s01", bufs=1, space="PSUM") as s01p,
            tc.tile_pool(name="o01", bufs=2, space="PSUM") as o01p,
            tc.tile_pool(name="pts1", bufs=2) as pts1p,
            tc.tile_pool(name="ptv1", bufs=2) as ptv1p,
            tc.tile_pool(name="ob1", bufs=2) as obp,
            tc.tile_pool(name="rs1", bufs=4) as rsbp,
            tc.tile_pool(name="ot1t", bufs=2) as ot1tp,
            tc.tile_pool(name="dscr", bufs=6, space="DRAM") as dscr_p,
        ):
            prev = None      # deferred (pts, ptv, half, ks, gsize, o_t0, o_t1)
            pend_fin = None  # deferred normalize for (o_t0, o_t1, j)

            def emit_mm2_l1(u):
                pts_t, ptv_t, half, ks, gsize, o_t0, o_t1 = u
                for t in range(gsize):
                    kc = ks + t
                    for h, o_t in ((0, o_t0), (1, o_t1)):
                        nc.tensor.matmul(
                            o_t[0:65, :], v_aug[h][:, kc, :],
                            p_slice(pts_t, ptv_t, half, (2 * t + h) * 512),
                            start=(kc == 0), stop=(kc == nk - 1),
                        )

            def finish_l1(u):
                o_t0, o_t1, j = u
                normalize(o_t0, ot01s[j][0:64, :], obp, rsbp, dscr_p)
                ot1 = ot1tp.tile([64, 512], md, tag="ot1")
                normalize(o_t1, ot1, obp, rsbp, dscr_p)
                nc.sync.dma_start(out=ot01s[j][64:128, :], in_=ot1)

            for j in range(nj):
                jsl = bass.ts(j, 512)
                o_t0 = o01p.tile([128, 512], F32, tag="o", name="ot0")
                o_t1 = o01p.tile([128, 512], F32, tag="o", name="ot1")
                ks = 0
                for gi, gsize in enumerate(groups1):
                    s01 = s01p.tile([128, 3072], F32, tag="s01")
                    for t in range(gsize):
                        kc = ks + t
                        # h0 on array rows 0:63, h1 on 64:127 — concurrent
                        nc.tensor.matmul(
                            s01[:, (2 * t) * 512:(2 * t + 1) * 512],
                            kt01[0:64, bass.ts(kc, 128)], qt01[0:64, jsl],
                            start=True, stop=True,
                        )
                        nc.tensor.matmul(
                            s01[:, (2 * t + 1) * 512:(2 * t + 2) * 512],
                            kt01[64:128, bass.ts(kc, 128)], qt01[64:128, jsl],
                            start=True, stop=True,
                        )
                    if prev is not None:
                        emit_mm2_l1(prev)
                        prev = None
                    half = gsize * 512
                    pts_t = pts1p.tile([128, 1536], md, tag="pts")
                    ptv_t = ptv1p.tile([128, 1536], mybir.dt.int16, tag="ptv")
                    emit_exp(s01, pts_t, ptv_t, 2 * half)
                    if pend_fin is not None:
                        finish_l1(pend_fin)
                        pend_fin = None
                    prev = (pts_t, ptv_t, half, ks, gsize, o_t0, o_t1)
                    ks += gsize
                # the last group's MM2 lands in next j's first block; o_t is
                # complete after it, so queue the normalize then.
                pend_fin = (o_t0, o_t1, j)
            if prev is not None:
                emit_mm2_l1(prev)
                prev = None
            if pend_fin is not None:
                finish_l1(pend_fin)
                pend_fin = None

        # ---- Loop 2: head h2 (even/odd chunk pairing) + outproj ----
        with (
            tc.tile_pool(name="s2", bufs=1, space="PSUM") as s2p,
            tc.tile_pool(name="o2", bufs=2, space="PSUM") as o2p,
            tc.tile_pool(name="yps", bufs=2, space="PSUM") as ypsp,
            tc.tile_pool(name="pts2", bufs=2) as pts2p,
            tc.tile_pool(name="ptv2", bufs=2) as ptv2p,
            tc.tile_pool(name="ob2", bufs=2) as obp2,
            tc.tile_pool(name="rs2", bufs=4) as rsbp2,
            tc.tile_pool(name="ytp", bufs=4) as ytp,
            tc.tile_pool(name="dscr2", bufs=4, space="DRAM") as dscr2_p,
        ):
            prev2 = None      # deferred (pts, ptv, g, o_t2)
            pend_fin2 = None  # deferred normalize (o_t2, j)
            op_q = []         # outproj cc-pair queue [(j, cc, cc2), ...]

            def emit_mm2_l2(u):
                pts_t, ptv_t, g, o_t2 = u
                for t in range(4):
                    kc = 4 * g + t
                    nc.tensor.matmul(
                        o_t2[0:65, :], v_aug[2][:, kc, :],
                        p_slice(pts_t, ptv_t, 1024, t * 512),
                        start=(kc == 0), stop=(kc == nk - 1),
                    )

            def finish_l2(u):
                o_t2, j = u
                normalize(o_t2, ot2ds[j][0:64, :], obp2, rsbp2, dscr2_p)
                nc.sync.dma_start(out=ot2ds[j][64:128, :], in_=ot2ds[j][0:64, :])

            def emit_op_pair(j, cc_a, cc_b):
                pjsl = bass.ts(j, 512)
                yps_a = ypsp.tile([128, 512], F32, tag="yps", name="ypsa")
                yps_b = ypsp.tile([128, 512], F32, tag="yps", name="ypsb")
                nc.tensor.matmul(yps_a, wp01_sb[:, bass.ts(cc_a, 128)],
                                 ot01s[j], start=True, stop=False)
                nc.tensor.matmul(yps_b, wp01_sb[:, bass.ts(cc_b, 128)],
                                 ot01s[j], start=True, stop=False)
                # the two K=64 wp2 parts run concurrently (rows 0:63 / 64:127)
                nc.tensor.matmul(yps_a, wp2d_sb[0:64, bass.ts(cc_a, 128)],
                                 ot2ds[j][0:64, :], start=False, stop=True)
                nc.tensor.matmul(yps_b, wp2d_sb[64:128, bass.ts(cc_b, 128)],
                                 ot2ds[j][64:128, :], start=False, stop=True)
                # alternate drain engine and DMA queue so the stores overlap
                for yps, cc, qeng in ((yps_a, cc_a, 0), (yps_b, cc_b, 1)):
                    yst = ytp.tile([128, 512], F32, tag="yt")
                    if qeng == 0:
                        nc.vector.tensor_copy(out=yst, in_=yps)
                        nc.sync.dma_start(out=yt_d[bass.ts(cc, 128), pjsl], in_=yst)
                    else:
                        nc.scalar.copy(out=yst, in_=yps)
                        nc.gpsimd.dma_start(out=yt_d[bass.ts(cc, 128), pjsl], in_=yst)

            for j in range(nj):
                jsl = bass.ts(j, 512)
                o_t2 = o2p.tile([128, 512], F32, tag="o2")
                if j > 0:
                    op_q.extend([(j - 1, 0, 1), (j - 1, 2, 3), (j - 1, 4, 5)])
                for g in range(ng2):
                    s2 = s2p.tile([128, 2048], F32, tag="s2")
                    for t in range(2):
                        kce = 4 * g + 2 * t
                        kco = kce + 1
                        nc.tensor.matmul(
                            s2[:, (2 * t) * 512:(2 * t + 1) * 512],
                            kt2d[0:64, bass.ts(kce, 128)], qt2d[0:64, jsl],
                            start=True, stop=True,
                        )
                        nc.tensor.matmul(
                            s2[:, (2 * t + 1) * 512:(2 * t + 2) * 512],
                            kt2d[64:128, bass.ts(kco, 128)], qt2d[64:128, jsl],
                            start=True, stop=True,
                        )
                    if prev2 is not None:
                        emit_mm2_l2(prev2)
                        prev2 = None
                    # spread the deferred output projection over late groups
                    if op_q and g >= max(1, ng2 - 5) and g % 2 == 1:
                        emit_op_pair(*op_q.pop(0))
                    pts_t = pts2p.tile([128, 1024], md, tag="pts2")
                    ptv_t = ptv2p.tile([128, 1024], mybir.dt.int16, tag="ptv2")
                    emit_exp(s2, pts_t, ptv_t, 2048)
                    if pend_fin2 is not None:
                        finish_l2(pend_fin2)
                        pend_fin2 = None
                    prev2 = (pts_t, ptv_t, g, o_t2)
                while op_q:
                    emit_op_pair(*op_q.pop(0))
                pend_fin2 = (o_t2, j)
            if prev2 is not None:
                emit_mm2_l2(prev2)
                prev2 = None
            if pend_fin2 is not None:
                finish_l2(pend_fin2)
                pend_fin2 = None
            for pair in ((nj - 1, 0, 1), (nj - 1, 2, 3), (nj - 1, 4, 5)):
                emit_op_pair(*pair)

        persist.release()
        consts.release()

    nc.compile()
    return nc


def make_core_inputs(x_b, w_qkv, w_proj, hbase, fast_mm=True):
    """Per-core input arrays for heads [hbase, hbase+3) of batch x_b."""
    C = DIM
    wq = [w_qkv[(hbase + h) * 64:(hbase + h + 1) * 64, :] * SCALE for h in range(3)]
    wk = [w_qkv[C + (hbase + h) * 64:C + (hbase + h + 1) * 64, :] for h in range(3)]
    wv = [w_qkv[2 * C + (hbase + h) * 64:2 * C + (hbase + h + 1) * 64, :] for h in range(3)]

    wqk = np.zeros((C, 384), np.float32)
    wqk[:, 0:64] = wq[0].T
    wqk[:, 64:128] = wq[1].T
    wqk[:, 128:192] = wk[0].T
    wqk[:, 192:256] = wk[1].T
    wqk[:, 256:320] = wq[2].T
    wqk[:, 320:384] = wk[2].T

    wv_p = np.zeros((C, 192), np.float32)
    for h in range(3):
        wv_p[:, h * 64:(h + 1) * 64] = wv[h].T

    wp = np.zeros((192, C), np.float32)
    for h in range(3):
        wp[h * 64:(h + 1) * 64, :] = w_proj[:, (hbase + h) * 64:(hbase + h + 1) * 64].T

    dt = ml_dtypes.bfloat16 if fast_mm else np.float32
    return {
        "xt": np.ascontiguousarray(x_b.T).astype(dt),
        "wqk": wqk.astype(dt),
        "wv": wv_p.astype(dt),
        "wp": wp.astype(dt),
    }


_NC_CACHE = {}


def get_nc(n=N_FULL, fast_mm=True, pace=True):
    key = (n, fast_mm, pace)
    if key not in _NC_CACHE:
        _NC_CACHE[key] = build_nc(n, fast_mm, pace)
    return _NC_CACHE[key]


def kernel(x, w_qkv, w_proj, b_proj, _trace=False):
    x = np.asarray(x, np.float32)
    w_qkv = np.asarray(w_qkv, np.float32)
    w_proj = np.asarray(w_proj, np.float32)
    b_proj = np.asarray(b_proj, np.float32)

    nc = get_nc(N_FULL, True)
    in_maps = []
    for c in range(N_CORES):
        b = c // 4
        hbase = (c % 4) * HEADS_PER_CORE
        in_maps.append(make_core_inputs(x[b], w_qkv, w_proj, hbase))

    res = run_bass_kernel_spmd(nc, in_maps, core_ids=list(range(N_CORES)),
                               trace=_trace)
    y = np.empty((B, N_FULL, DIM), np.float32)
    for b in range(B):
        acc = res.results[4 * b]["yt"].astype(np.float32)
        for c in range(4 * b + 1, 4 * b + 4):
            acc = acc + res.results[c]["yt"]
        y[b] = acc.T + b_proj[None, :]
    if _trace:
        return y, res
    return y


# revision 20
# speedup vs baseline: 1.2918x; 1.2918x over previous
"""Multi-head attention (B=2, N=4096, C=768, H=12, D=64) on 8 TRN2 NeuronCores.

Sharding: tensor-parallel over (batch, head). B*H = 24 pairs -> 3 per core.
Cores 0-3 handle batch 0, cores 4-7 batch 1 (3 consecutive heads each).
Each core computes the QKV projection, attention, and a partial output
projection for its heads, returning a partial y^T [768, 4096]. The host
sums the 4 partials per batch, transposes, and adds the bias.

Key performance structure (v2, vs the paced baseline):

1. MM1 (S^T = K_chunk @ q^T) has contraction D=64 — half the PE array.
   Both 64-row halves of the array run CONCURRENTLY via tile_position row
   tiling (auto-derived from base partitions): heads h0/h1 are packed on
   partitions 0:64 / 64:128 of qt01/kt01, so emitting their chunk-matmuls
   back-to-back makes them co-execute (2x MM1 throughput). Head h2 is
   paired with itself across even/odd k-chunks, using q2/k2 duplicated on
   both partition halves (qt2d/kt2d).
2. The softmax exp (50M elements/core) saturates ScalarE (~327us alone),
   so it is split: ScalarE does true exp on the leading column span of
   each PSUM score group; the DVE computes a Schraudolph bit-trick exp
   (int32(s*A+B) reinterpreted as fp32) on the rest in a single
   tensor_scalar pass. P is stored fp32; MM2 runs with float32r operands
   (same PE rate as bf16 at N=512).
3. MM2 keeps the ones-column trick (v_aug M=65) for softmax denominators.
4. Output projection: wp01 part is K=128; the wp2 (K=64) parts of two
   adjacent output chunks are row-tiled concurrently using wp2/ot2
   duplicated on both partition halves.

Per-j schedule, heads h0+h1 (loop1): groups of 3 k-chunks; per group the
PE does [3 MM1-pair slots][6 deferred MM2 of prev group] while ScalarE+DVE
exp the previous S banks; S PSUM single-buffered [128,3072] (6 banks) +
o_t0/o_t1 (2 banks) = 16KB exactly. Loop2 does h2 (groups of 4 chunks via
even/odd pairing) plus the deferred output projection of the previous j.
"""

import os

import ml_dtypes
import numpy as np

import concourse.bass as bass
import concourse.mybir as mybir
import concourse.tile as tile
from concourse import bacc
from concourse.bass_utils import run_bass_kernel_spmd

F32 = mybir.dt.float32
F32R = mybir.dt.float32r
I32 = mybir.dt.int32
BF16 = mybir.dt.bfloat16

DIM = 768
NUM_HEADS = 12
HEAD_DIM = 64
SCALE = HEAD_DIM ** -0.5
B = 2
N_FULL = 4096
N_CORES = 8
HEADS_PER_CORE = 3
CC = DIM // 128  # 6 contraction chunks

# Schraudolph fast-exp constants, bf16 variant:
# exp(s) ~= bitcast_bf16(int16(s*EXPA + EXPB)). bf16 is the top half of
# fp32, so the exponent field sits at bit 7. EXPB centers the sawtooth
# error (~+-3.4% max) and includes rounding compensation for the fp->int
# truncation (values are always positive here).
EXPA = 184.6650292  # 2^7 / ln(2)
EXPB = 16250.5      # 127*2^7 - centering

# Column split of each exp group between ScalarE (true exp, [0:SC)) and
# DVE (bit-trick, [SC:end)). Tuned so each engine stays under the PE time.
SC1_COLS = 1792  # of 3072 (loop1 full groups)
SC2_COLS = 1280  # of 2048 (loop2 groups)


def build_nc(n=N_FULL, fast_mm=True, pace=True):
    """Build the per-core Bass program. Same program runs SPMD on all
    cores; per-core inputs differ (x^T batch + per-head weight slices)."""
    nj = n // 512      # q slices
    nk = n // 128      # k chunks
    md = BF16 if fast_mm else F32R

    nc = bacc.Bacc("TRN2", target_bir_lowering=False, debug=False)

    xt_d = nc.dram_tensor("xt", [DIM, n], md, kind="ExternalInput")
    wqk_d = nc.dram_tensor("wqk", [DIM, 384], md, kind="ExternalInput")
    wv_d = nc.dram_tensor("wv", [DIM, 192], md, kind="ExternalInput")
    wp_d = nc.dram_tensor("wp", [192, DIM], md, kind="ExternalInput")
    yt_d = nc.dram_tensor("yt", [DIM, n], F32, kind="ExternalOutput")

    # loop1 (h0+h1) k-chunk groups of 3; loop2 (h2) groups of 4.
    groups1 = [3] * (nk // 3)
    if nk % 3:
        groups1.append(nk % 3)
    ng2 = nk // 4

    lp = nc.allow_low_precision(
        reason="bf16 matmul operands; fp32 PSUM accumulation; fast-exp "
               "bit trick on part of the softmax within tolerance")
    with lp, tile.TileContext(nc) as tc:
        consts = tc.alloc_tile_pool(name="consts", bufs=1)
        persist = tc.alloc_tile_pool(name="persist", bufs=1)

        wqk_sb = consts.tile([128, CC, 384], md, tag="wqk")
        wqk_r = wqk_d[:, :].rearrange("(a p) m -> p a m", p=128)
        nc.sync.dma_start(out=wqk_sb[:, 0:2, :], in_=wqk_r[:, 0:2, :])
        nc.sync.dma_start(out=wqk_sb[:, 2:CC, :], in_=wqk_r[:, 2:CC, :])
        wv_sb = consts.tile([128, CC, 192], md, tag="wv")
        nc.sync.dma_start(out=wv_sb, in_=wv_d[:, :].rearrange("(a p) m -> p a m", p=128))
        wp01_sb = consts.tile([128, DIM], md, tag="wp01")
        nc.gpsimd.dma_start(out=wp01_sb, in_=wp_d[0:128, :])
        # wp2 duplicated on both partition halves for row-tiled outproj
        wp2d_sb = consts.tile([128, DIM], md, tag="wp2d")
        nc.gpsimd.dma_start(out=wp2d_sb[0:64, :], in_=wp_d[128:192, :])
        nc.gpsimd.dma_start(out=wp2d_sb[64:128, :], in_=wp_d[128:192, :])

        # Persistent activations.
        qt01 = persist.tile([128, n], md, tag="qt01")  # parts 0:64 h0, 64:128 h1
        kt01 = persist.tile([128, n], md, tag="kt01")
        qt2d = persist.tile([128, n], md, tag="qt2d")  # q2 duplicated both halves
        kt2d = persist.tile([128, n], md, tag="kt2d")  # k2 duplicated both halves
        v_aug = [persist.tile([128, nk, 65], md, tag=f"vaug{h}", name=f"vaug{h}")
                 for h in range(HEADS_PER_CORE)]
        for h in range(HEADS_PER_CORE):
            ones_col = v_aug[h][:, :, 64:65]
            if md == F32R:
                ones_col = ones_col.bitcast(F32)
            nc.vector.memset(ones_col, 1.0)
        # Normalized per-head outputs, persist until the output projection.
        ot01s = [persist.tile([128, 512], md, tag=f"ot01_{j}", name=f"ot01_{j}")
                 for j in range(nj)]
        ot2ds = [persist.tile([128, 512], md, tag=f"ot2d_{j}", name=f"ot2d_{j}")
                 for j in range(nj)]

        # Preload the exp table set while ScalarE is otherwise idle so the
        # ~2.7us ACT_TABLE_LOAD doesn't gate the first attention group.
        actwarm = consts.tile([1, 2], F32, tag="actwarm")
        nc.vector.memset(actwarm, 0.0)
        nc.scalar.activation(out=actwarm[0:1, 1:2], in_=actwarm[0:1, 0:1],
                             func=mybir.ActivationFunctionType.Exp)

        # ---- Phase A: QKV projections from pre-transposed x ----
        with (
            tc.tile_pool(name="xtj", bufs=2) as xtj_p,
            tc.tile_pool(name="qk_ps", bufs=1, space="PSUM") as qk_ps,
            tc.tile_pool(name="v_ps", bufs=1, space="PSUM") as v_ps,
        ):
            for j in range(nj):
                jsl = bass.ts(j, 512)
                xtj = xtj_p.tile([128, CC, 512], md, tag="xtj")
                xt_r = xt_d[:, jsl].rearrange("(a p) m -> p a m", p=128)
                if j == 0:
                    nc.sync.dma_start(out=xtj[:, 0:2, :], in_=xt_r[:, 0:2, :])
                    nc.sync.dma_start(out=xtj[:, 2:CC, :], in_=xt_r[:, 2:CC, :])
                else:
                    nc.sync.dma_start(out=xtj, in_=xt_r)
                # q/k projections: packs [q0|q1], [k0|k1], [q2|k2]
                for pi, colbase in enumerate((0, 128, 256)):
                    ps = qk_ps.tile([128, 512], F32, tag="qk")
                    for cc in range(CC):
                        nc.tensor.matmul(
                            ps,
                            wqk_sb[:, cc, colbase:colbase + 128],
                            xtj[:, cc, :],
                            start=(cc == 0), stop=(cc == CC - 1),
                        )
                    if pi == 0:
                        nc.vector.tensor_copy(out=qt01[:, j * 512:j * 512 + 256], in_=ps[:, 0:256])
                        nc.vector.tensor_copy(out=qt01[:, j * 512 + 256:j * 512 + 512], in_=ps[:, 256:512])
                    elif pi == 1:
                        nc.vector.tensor_copy(out=kt01[:, j * 512:j * 512 + 256], in_=ps[:, 0:256])
                        nc.vector.tensor_copy(out=kt01[:, j * 512 + 256:j * 512 + 512], in_=ps[:, 256:512])
                    else:
                        # q2 -> lo directly, k2 -> hi directly; then DMA
                        # partition-shifts create the duplicated halves.
                        nc.vector.tensor_copy(out=qt2d[0:64, jsl], in_=ps[0:64, :])
                        nc.vector.tensor_copy(out=kt2d[64:128, jsl], in_=ps[64:128, :])
                        nc.sync.dma_start(out=qt2d[64:128, jsl], in_=qt2d[0:64, jsl])
                        nc.sync.dma_start(out=kt2d[0:64, jsl], in_=kt2d[64:128, jsl])

                # v projection (natural orientation), 3 heads
                for rc in range(4):
                    ps = v_ps.tile([128, 192], F32, tag="v")
                    for cc in range(CC):
                        nc.tensor.matmul(
                            ps,
                            xtj[:, cc, bass.ts(rc, 128)],
                            wv_sb[:, cc, :],
                            start=(cc == 0), stop=(cc == CC - 1),
                        )
                    kc = j * 4 + rc
                    for h in range(HEADS_PER_CORE):
                        nc.vector.tensor_copy(
                            out=v_aug[h][:, kc, 0:64], in_=ps[:, bass.ts(h, 64)]
                        )

        exp_dve = os.environ.get("EXP_DVE", "1") == "1"

        def emit_exp(sa_t, sb_t, pts_t, ptv_t, half):
            """exp of a score group split across two PSUM tiles (sa for
            ScalarE true exp -> pts bf16, sb for the DVE bit-trick -> ptv
            int16 holding bf16 bit patterns). Separate PSUM tiles matter:
            reads of one PSUM tile are serialized by the dep tracker, so a
            shared tile would chain DVE behind ScalarE."""
            nc.scalar.activation(
                out=pts_t[:, 0:half], in_=sa_t[:, 0:half],
                func=mybir.ActivationFunctionType.Exp,
            )
            if exp_dve:
                nc.vector.tensor_scalar(
                    out=ptv_t[:, 0:half],
                    in0=sb_t[:, 0:half],
                    scalar1=float(EXPA), scalar2=float(EXPB),
                    op0=mybir.AluOpType.mult, op1=mybir.AluOpType.add,
                )
            else:
                nc.scalar.activation(
                    out=ptv_t[:, 0:half].bitcast(md),
                    in_=sb_t[:, 0:half],
                    func=mybir.ActivationFunctionType.Exp,
                )

        def p_slice(pts_t, ptv_t, half, c):
            """MM2 rhs for the 512-col chunk at interleaved column c."""
            if c < half:
                return pts_t[:, c:c + 512]
            return ptv_t[:, c - half:c - half + 512].bitcast(md)

        def normalize(o_t, dst64, obp, rsbp, dscr_p):
            """Drain o_t [65,512] PSUM, divide rows 0:64 by row 64, write
            the normalized bf16 result to dst64 (a [64,512]-shaped AP on
            partitions 0:64). Returns nothing; all off the PE."""
            ob = obp.tile([128, 512], F32, tag="ob")
            nc.vector.tensor_copy(out=ob[0:65, :], in_=o_t[0:65, :])
            scr = dscr_p.tile([512], F32, tag="scr")
            nc.sync.dma_start(out=scr, in_=ob[64:65, :])
            r4 = rsbp.tile([128, 4], F32, tag="r4")
            nc.sync.dma_start(out=r4, in_=scr.rearrange("(p f) -> p f", p=128))
            r4i = rsbp.tile([128, 4], F32, tag="r4i")
            nc.vector.reciprocal(out=r4i, in_=r4)
            scr2 = dscr_p.tile([512], F32, tag="scr2")
            nc.sync.dma_start(out=scr2, in_=r4i)
            bcs = rsbp.tile([64, 512], F32, tag="bcs")
            scr_b = bass.AP(tensor=scr2.tensor, offset=scr2.offset,
                            ap=[[0, 64]] + list(scr2.ap))
            nc.sync.dma_start(out=bcs, in_=scr_b)
            nc.vector.tensor_mul(dst64, ob[0:64, :], bcs)

        # ---- Loop 1: heads h0+h1 attention (row-tiled MM1 pairs) ----
        # S layout interleaves heads by slot: slot t writes h0 -> col 2t*512,
        # h1 -> (2t+1)*512, so ScalarE's half of the group is complete early.
        with (
            tc.tile_pool(name="s01a", bufs=1, space="PSUM") as s01ap,
            tc.tile_pool(name="s01b", bufs=1, space="PSUM") as s01bp,
            tc.tile_pool(name="o01", bufs=2, space="PSUM") as o01p,
            tc.tile_pool(name="pts1", bufs=2) as pts1p,
            tc.tile_pool(name="ptv1", bufs=2) as ptv1p,
            tc.tile_pool(name="ob1", bufs=2) as obp,
            tc.tile_pool(name="rs1", bufs=4) as rsbp,
            tc.tile_pool(name="ot1t", bufs=2) as ot1tp,
            tc.tile_pool(name="dscr", bufs=6, space="DRAM") as dscr_p,
        ):
            prev = None      # deferred (pts, ptv, half, ks, gsize, o_t0, o_t1)
            pend_fin = None  # deferred normalize for (o_t0, o_t1, j)

            def emit_mm2_l1(u):
                pts_t, ptv_t, half, ks, gsize, o_t0, o_t1 = u
                for t in range(gsize):
                    kc = ks + t
                    for h, o_t in ((0, o_t0), (1, o_t1)):
                        nc.tensor.matmul(
                            o_t[0:65, :], v_aug[h][:, kc, :],
                            p_slice(pts_t, ptv_t, half, (2 * t + h) * 512),
                            start=(kc == 0), stop=(kc == nk - 1),
                        )

            def finish_l1(u):
                o_t0, o_t1, j = u
                normalize(o_t0, ot01s[j][0:64, :], obp, rsbp, dscr_p)
                ot1 = ot1tp.tile([64, 512], md, tag="ot1")
                normalize(o_t1, ot1, obp, rsbp, dscr_p)
                nc.sync.dma_start(out=ot01s[j][64:128, :], in_=ot1)

            for j in range(nj):
                jsl = bass.ts(j, 512)
                o_t0 = o01p.tile([128, 512], F32, tag="o", name="ot0")
                o_t1 = o01p.tile([128, 512], F32, tag="o", name="ot1")
                ks = 0
                for gi, gsize in enumerate(groups1):
                    s_a = s01ap.tile([128, 1536], F32, tag="sa")
                    s_b = s01bp.tile([128, 1536], F32, tag="sb")
                    half = gsize * 512
                    for t in range(gsize):
                        kc = ks + t
                        for h in range(2):
                            c = (2 * t + h) * 512
                            dst = (s_a[:, c:c + 512] if c < half
                                   else s_b[:, c - half:c - half + 512])
                            # h0 on array rows 0:63, h1 on 64:127 — concurrent
                            nc.tensor.matmul(
                                dst,
                                kt01[h * 64:(h + 1) * 64, bass.ts(kc, 128)],
                                qt01[h * 64:(h + 1) * 64, jsl],
                                start=True, stop=True,
                            )
                    if prev is not None:
                        emit_mm2_l1(prev)
                        prev = None
                    pts_t = pts1p.tile([128, 1536], md, tag="pts")
                    ptv_t = ptv1p.tile([128, 1536], mybir.dt.int16, tag="ptv")
                    emit_exp(s_a, s_b, pts_t, ptv_t, half)
                    if pend_fin is not None:
                        finish_l1(pend_fin)
                        pend_fin = None
                    prev = (pts_t, ptv_t, half, ks, gsize, o_t0, o_t1)
                    ks += gsize
                # the last group's MM2 lands in next j's first block; o_t is
                # complete after it, so queue the normalize then.
                pend_fin = (o_t0, o_t1, j)
            if prev is not None:
                emit_mm2_l1(prev)
                prev = None
            if pend_fin is not None:
                finish_l1(pend_fin)
                pend_fin = None

        # ---- Loop 2: head h2 (even/odd chunk pairing) + outproj ----
        with (
            tc.tile_pool(name="s2a", bufs=1, space="PSUM") as s2ap,
            tc.tile_pool(name="s2b", bufs=1, space="PSUM") as s2bp,
            tc.tile_pool(name="o2", bufs=2, space="PSUM") as o2p,
            tc.tile_pool(name="yps", bufs=2, space="PSUM") as ypsp,
            tc.tile_pool(name="pts2", bufs=2) as pts2p,
            tc.tile_pool(name="ptv2", bufs=2) as ptv2p,
            tc.tile_pool(name="ob2", bufs=2) as obp2,
            tc.tile_pool(name="rs2", bufs=4) as rsbp2,
            tc.tile_pool(name="ytp", bufs=4) as ytp,
            tc.tile_pool(name="dscr2", bufs=4, space="DRAM") as dscr2_p,
        ):
            prev2 = None      # deferred (pts, ptv, g, o_t2)
            pend_fin2 = None  # deferred normalize (o_t2, j)
            op_q = []         # outproj cc-pair queue [(j, cc, cc2), ...]

            def emit_mm2_l2(u):
                pts_t, ptv_t, g, o_t2 = u
                for t in range(4):
                    kc = 4 * g + t
                    nc.tensor.matmul(
                        o_t2[0:65, :], v_aug[2][:, kc, :],
                        p_slice(pts_t, ptv_t, 1024, t * 512),
                        start=(kc == 0), stop=(kc == nk - 1),
                    )

            def finish_l2(u):
                o_t2, j = u
                normalize(o_t2, ot2ds[j][0:64, :], obp2, rsbp2, dscr2_p)
                nc.sync.dma_start(out=ot2ds[j][64:128, :], in_=ot2ds[j][0:64, :])

            def emit_op_pair(j, cc_a, cc_b):
                pjsl = bass.ts(j, 512)
                yps_a = ypsp.tile([128, 512], F32, tag="yps", name="ypsa")
                yps_b = ypsp.tile([128, 512], F32, tag="yps", name="ypsb")
                nc.tensor.matmul(yps_a, wp01_sb[:, bass.ts(cc_a, 128)],
                                 ot01s[j], start=True, stop=False)
                nc.tensor.matmul(yps_b, wp01_sb[:, bass.ts(cc_b, 128)],
                                 ot01s[j], start=True, stop=False)
                # the two K=64 wp2 parts run concurrently (rows 0:63 / 64:127)
                nc.tensor.matmul(yps_a, wp2d_sb[0:64, bass.ts(cc_a, 128)],
                                 ot2ds[j][0:64, :], start=False, stop=True)
                nc.tensor.matmul(yps_b, wp2d_sb[64:128, bass.ts(cc_b, 128)],
                                 ot2ds[j][64:128, :], start=False, stop=True)
                # alternate drain engine and DMA queue so the stores overlap
                for yps, cc, qeng in ((yps_a, cc_a, 0), (yps_b, cc_b, 1)):
                    yst = ytp.tile([128, 512], F32, tag="yt")
                    if qeng == 0:
                        nc.vector.tensor_copy(out=yst, in_=yps)
                        nc.sync.dma_start(out=yt_d[bass.ts(cc, 128), pjsl], in_=yst)
                    else:
                        nc.scalar.copy(out=yst, in_=yps)
                        nc.gpsimd.dma_start(out=yt_d[bass.ts(cc, 128), pjsl], in_=yst)

            for j in range(nj):
                jsl = bass.ts(j, 512)
                o_t2 = o2p.tile([128, 512], F32, tag="o2")
                if j > 0:
                    op_q.extend([(j - 1, 0, 1), (j - 1, 2, 3), (j - 1, 4, 5)])
                for g in range(ng2):
                    s_a = s2ap.tile([128, 1024], F32, tag="sa2")
                    s_b = s2bp.tile([128, 1024], F32, tag="sb2")
                    for t in range(4):
                        # chunk 4g+t at interleaved col t*512; t<2 -> s_a
                        kc = 4 * g + t
                        c = (t % 2) * 512
                        dst = s_a[:, c:c + 512] if t < 2 else s_b[:, c:c + 512]
                        hb = (t % 2) * 64
                        nc.tensor.matmul(
                            dst,
                            kt2d[hb:hb + 64, bass.ts(kc, 128)],
                            qt2d[hb:hb + 64, jsl],
                            start=True, stop=True,
                        )
                    if prev2 is not None:
                        emit_mm2_l2(prev2)
                        prev2 = None
                    # spread the deferred output projection over late groups
                    if op_q and g >= max(1, ng2 - 5) and g % 2 == 1:
                        emit_op_pair(*op_q.pop(0))
                    pts_t = pts2p.tile([128, 1024], md, tag="pts2")
                    ptv_t = ptv2p.tile([128, 1024], mybir.dt.int16, tag="ptv2")
                    emit_exp(s_a, s_b, pts_t, ptv_t, 1024)
                    if pend_fin2 is not None:
                        finish_l2(pend_fin2)
                        pend_fin2 = None
                    prev2 = (pts_t, ptv_t, g, o_t2)
                while op_q:
                    emit_op_pair(*op_q.pop(0))
                pend_fin2 = (o_t2, j)
            if prev2 is not None:
                emit_mm2_l2(prev2)
                prev2 = None
            if pend_fin2 is not None:
                finish_l2(pend_fin2)
                pend_fin2 = None
            for pair in ((nj - 1, 0, 1), (nj - 1, 2, 3), (nj - 1, 4, 5)):
                emit_op_pair(*pair)

        persist.release()
        consts.release()

    nc.compile()
    return nc


def make_core_inputs(x_b, w_qkv, w_proj, hbase, fast_mm=True):
    """Per-core input arrays for heads [hbase, hbase+3) of batch x_b."""
    C = DIM
    wq = [w_qkv[(hbase + h) * 64:(hbase + h + 1) * 64, :] * SCALE for h in range(3)]
    wk = [w_qkv[C + (hbase + h) * 64:C + (hbase + h + 1) * 64, :] for h in range(3)]
    wv = [w_qkv[2 * C + (hbase + h) * 64:2 * C + (hbase + h + 1) * 64, :] for h in range(3)]

    wqk = np.zeros((C, 384), np.float32)
    wqk[:, 0:64] = wq[0].T
    wqk[:, 64:128] = wq[1].T
    wqk[:, 128:192] = wk[0].T
    wqk[:, 192:256] = wk[1].T
    wqk[:, 256:320] = wq[2].T
    wqk[:, 320:384] = wk[2].T

    wv_p = np.zeros((C, 192), np.float32)
    for h in range(3):
        wv_p[:, h * 64:(h + 1) * 64] = wv[h].T

    wp = np.zeros((192, C), np.float32)
    for h in range(3):
        wp[h * 64:(h + 1) * 64, :] = w_proj[:, (hbase + h) * 64:(hbase + h + 1) * 64].T

    dt = ml_dtypes.bfloat16 if fast_mm else np.float32
    return {
        "xt": np.ascontiguousarray(x_b.T).astype(dt),
        "wqk": wqk.astype(dt),
        "wv": wv_p.astype(dt),
        "wp": wp.astype(dt),
    }


_NC_CACHE = {}


def get_nc(n=N_FULL, fast_mm=True, pace=True):
    key = (n, fast_mm, pace)
    if key not in _NC_CACHE:
        _NC_CACHE[key] = build_nc(n, fast_mm, pace)
    return _NC_CACHE[key]


def kernel(x, w_qkv, w_proj, b_proj, _trace=False):
    x = np.asarray(x, np.float32)
    w_qkv = np.asarray(w_qkv, np.float32)
    w_proj = np.asarray(w_proj, np.float32)
    b_proj = np.asarray(b_proj, np.float32)

    nc = get_nc(N_FULL, True)
    in_maps = []
    for c in range(N_CORES):
        b = c // 4
        hbase = (c % 4) * HEADS_PER_CORE
        in_maps.append(make_core_inputs(x[b], w_qkv, w_proj, hbase))

    res = run_bass_kernel_spmd(nc, in_maps, core_ids=list(range(N_CORES)),
                               trace=_trace)
    y = np.empty((B, N_FULL, DIM), np.float32)
    for b in range(B):
        acc = res.results[4 * b]["yt"].astype(np.float32)
        for c in range(4 * b + 1, 4 * b + 4):
            acc = acc + res.results[c]["yt"]
        y[b] = acc.T + b_proj[None, :]
    if _trace:
        return y, res
    return y


# revision 25
# speedup vs baseline: 1.4683x; 1.1366x over previous
"""Multi-head attention (B=2, N=4096, C=768, H=12, D=64) on 8 TRN2 NeuronCores.

Sharding: tensor-parallel over (batch, head). B*H = 24 pairs -> 3 per core.
Cores 0-3 handle batch 0, cores 4-7 batch 1 (3 consecutive heads each).
Each core computes the QKV projection, attention, and a partial output
projection for its heads, returning a partial y^T [768, 4096]. The host
sums the 4 partials per batch, transposes, and adds the bias.

Key performance structure (v2, vs the paced baseline):

1. MM1 (S^T = K_chunk @ q^T) has contraction D=64 — half the PE array.
   Both 64-row halves of the array run CONCURRENTLY via tile_position row
   tiling (auto-derived from base partitions): heads h0/h1 are packed on
   partitions 0:64 / 64:128 of qt01/kt01, so emitting their chunk-matmuls
   back-to-back makes them co-execute (2x MM1 throughput). Head h2 is
   paired with itself across even/odd k-chunks, using q2/k2 duplicated on
   both partition halves (qt2d/kt2d).
2. The softmax exp (50M elements/core) saturates ScalarE (~327us alone),
   so it is split: ScalarE does true exp on the leading column span of
   each PSUM score group; the DVE computes a Schraudolph bit-trick exp
   (int32(s*A+B) reinterpreted as fp32) on the rest in a single
   tensor_scalar pass. P is stored fp32; MM2 runs with float32r operands
   (same PE rate as bf16 at N=512).
3. MM2 keeps the ones-column trick (v_aug M=65) for softmax denominators.
4. Output projection: wp01 part is K=128; the wp2 (K=64) parts of two
   adjacent output chunks are row-tiled concurrently using wp2/ot2
   duplicated on both partition halves.

Per-j schedule, heads h0+h1 (loop1): groups of 3 k-chunks; per group the
PE does [3 MM1-pair slots][6 deferred MM2 of prev group] while ScalarE+DVE
exp the previous S banks; S PSUM single-buffered [128,3072] (6 banks) +
o_t0/o_t1 (2 banks) = 16KB exactly. Loop2 does h2 (groups of 4 chunks via
even/odd pairing) plus the deferred output projection of the previous j.
"""

import os

import ml_dtypes
import numpy as np

import concourse.bass as bass
import concourse.mybir as mybir
import concourse.tile as tile
from concourse import bacc
from concourse.bass_utils import run_bass_kernel_spmd

F32 = mybir.dt.float32
F32R = mybir.dt.float32r
I32 = mybir.dt.int32
BF16 = mybir.dt.bfloat16

DIM = 768
NUM_HEADS = 12
HEAD_DIM = 64
SCALE = HEAD_DIM ** -0.5
B = 2
N_FULL = 4096
N_CORES = 8
HEADS_PER_CORE = 3
CC = DIM // 128  # 6 contraction chunks

# Schraudolph fast-exp constants, bf16 variant:
# exp(s) ~= bitcast_bf16(int16(s*EXPA + EXPB)). bf16 is the top half of
# fp32, so the exponent field sits at bit 7. EXPB centers the sawtooth
# error (~+-3.4% max) and includes rounding compensation for the fp->int
# truncation (values are always positive here).
EXPA = 184.6650292  # 2^7 / ln(2)
EXPB = 16250.5      # 127*2^7 - centering

# Column split of each exp group between ScalarE (true exp, [0:SC)) and
# DVE (bit-trick, [SC:end)). Tuned so each engine stays under the PE time.
SC1_COLS = 1792  # of 3072 (loop1 full groups)
SC2_COLS = 1280  # of 2048 (loop2 groups)


def build_nc(n=N_FULL, fast_mm=True, pace=True):
    """Build the per-core Bass program. Same program runs SPMD on all
    cores; per-core inputs differ (x^T batch + per-head weight slices)."""
    nj = n // 512      # q slices
    nk = n // 128      # k chunks
    md = BF16 if fast_mm else F32R

    nc = bacc.Bacc("TRN2", target_bir_lowering=False, debug=False)

    xt_d = nc.dram_tensor("xt", [DIM, n], md, kind="ExternalInput")
    wqk_d = nc.dram_tensor("wqk", [DIM, 384], md, kind="ExternalInput")
    wv_d = nc.dram_tensor("wv", [DIM, 192], md, kind="ExternalInput")
    wp_d = nc.dram_tensor("wp", [192, DIM], md, kind="ExternalInput")
    yt_d = nc.dram_tensor("yt", [DIM, n], F32, kind="ExternalOutput")

    # loop1 (h0+h1) k-chunk groups of 3; loop2 (h2) groups of 4.
    groups1 = [3] * (nk // 3)
    if nk % 3:
        groups1.append(nk % 3)
    ng2 = nk // 4

    lp = nc.allow_low_precision(
        reason="bf16 matmul operands; fp32 PSUM accumulation; fast-exp "
               "bit trick on part of the softmax within tolerance")
    with lp, tile.TileContext(nc) as tc:
        consts = tc.alloc_tile_pool(name="consts", bufs=1)
        persist = tc.alloc_tile_pool(name="persist", bufs=1)

        wqk_sb = consts.tile([128, CC, 384], md, tag="wqk")
        wqk_r = wqk_d[:, :].rearrange("(a p) m -> p a m", p=128)
        nc.sync.dma_start(out=wqk_sb[:, 0:2, :], in_=wqk_r[:, 0:2, :])
        nc.sync.dma_start(out=wqk_sb[:, 2:CC, :], in_=wqk_r[:, 2:CC, :])
        wv_sb = consts.tile([128, CC, 192], md, tag="wv")
        nc.sync.dma_start(out=wv_sb, in_=wv_d[:, :].rearrange("(a p) m -> p a m", p=128))
        wp01_sb = consts.tile([128, DIM], md, tag="wp01")
        nc.gpsimd.dma_start(out=wp01_sb, in_=wp_d[0:128, :])
        # wp2 duplicated on both partition halves for row-tiled outproj
        wp2d_sb = consts.tile([128, DIM], md, tag="wp2d")
        nc.gpsimd.dma_start(out=wp2d_sb[0:64, :], in_=wp_d[128:192, :])
        nc.gpsimd.dma_start(out=wp2d_sb[64:128, :], in_=wp_d[128:192, :])

        # Persistent activations.
        qt01 = persist.tile([128, n], md, tag="qt01")  # parts 0:64 h0, 64:128 h1
        kt01 = persist.tile([128, n], md, tag="kt01")
        qt2d = persist.tile([128, n], md, tag="qt2d")  # q2 duplicated both halves
        kt2d = persist.tile([128, n], md, tag="kt2d")  # k2 duplicated both halves
        v_aug = [persist.tile([128, nk, 65], md, tag=f"vaug{h}", name=f"vaug{h}")
                 for h in range(HEADS_PER_CORE)]
        for h in range(HEADS_PER_CORE):
            ones_col = v_aug[h][:, :, 64:65]
            if md == F32R:
                ones_col = ones_col.bitcast(F32)
            nc.vector.memset(ones_col, 1.0)
        # Normalized per-head outputs, persist until the output projection.
        ot01s = [persist.tile([128, 512], md, tag=f"ot01_{j}", name=f"ot01_{j}")
                 for j in range(nj)]
        ot2ds = [persist.tile([128, 512], md, tag=f"ot2d_{j}", name=f"ot2d_{j}")
                 for j in range(nj)]

        # Preload the exp table set while ScalarE is otherwise idle so the
        # ~2.7us ACT_TABLE_LOAD doesn't gate the first attention group.
        actwarm = consts.tile([1, 2], F32, tag="actwarm")
        nc.vector.memset(actwarm, 0.0)
        nc.scalar.activation(out=actwarm[0:1, 1:2], in_=actwarm[0:1, 0:1],
                             func=mybir.ActivationFunctionType.Exp)

        # ---- Phase A: QKV projections from pre-transposed x ----
        with (
            tc.tile_pool(name="xtj", bufs=2) as xtj_p,
            tc.tile_pool(name="qk_ps", bufs=2, space="PSUM") as qk_ps,
            tc.tile_pool(name="v_ps", bufs=2, space="PSUM") as v_ps,
        ):
            for j in range(nj):
                jsl = bass.ts(j, 512)
                xtj = xtj_p.tile([128, CC, 512], md, tag="xtj")
                xt_r = xt_d[:, jsl].rearrange("(a p) m -> p a m", p=128)
                if j == 0:
                    nc.sync.dma_start(out=xtj[:, 0:2, :], in_=xt_r[:, 0:2, :])
                    nc.sync.dma_start(out=xtj[:, 2:CC, :], in_=xt_r[:, 2:CC, :])
                else:
                    nc.sync.dma_start(out=xtj, in_=xt_r)
                # q/k projections: packs [q0|q1], [k0|k1], [q2|k2]
                for pi, colbase in enumerate((0, 128, 256)):
                    ps = qk_ps.tile([128, 512], F32, tag="qk")
                    for cc in range(CC):
                        nc.tensor.matmul(
                            ps,
                            wqk_sb[:, cc, colbase:colbase + 128],
                            xtj[:, cc, :],
                            start=(cc == 0), stop=(cc == CC - 1),
                        )
                    if pi == 0:
                        nc.vector.tensor_copy(out=qt01[:, j * 512:j * 512 + 256], in_=ps[:, 0:256])
                        nc.vector.tensor_copy(out=qt01[:, j * 512 + 256:j * 512 + 512], in_=ps[:, 256:512])
                    elif pi == 1:
                        nc.vector.tensor_copy(out=kt01[:, j * 512:j * 512 + 256], in_=ps[:, 0:256])
                        nc.vector.tensor_copy(out=kt01[:, j * 512 + 256:j * 512 + 512], in_=ps[:, 256:512])
                    else:
                        # q2 -> lo directly, k2 -> hi directly; then DMA
                        # partition-shifts create the duplicated halves.
                        nc.vector.tensor_copy(out=qt2d[0:64, jsl], in_=ps[0:64, :])
                        nc.vector.tensor_copy(out=kt2d[64:128, jsl], in_=ps[64:128, :])
                        nc.sync.dma_start(out=qt2d[64:128, jsl], in_=qt2d[0:64, jsl])
                        nc.sync.dma_start(out=kt2d[0:64, jsl], in_=kt2d[64:128, jsl])

                # v projection (natural orientation), 3 heads
                for rc in range(4):
                    ps = v_ps.tile([128, 192], F32, tag="v")
                    for cc in range(CC):
                        nc.tensor.matmul(
                            ps,
                            xtj[:, cc, bass.ts(rc, 128)],
                            wv_sb[:, cc, :],
                            start=(cc == 0), stop=(cc == CC - 1),
                        )
                    kc = j * 4 + rc
                    for h in range(HEADS_PER_CORE):
                        nc.vector.tensor_copy(
                            out=v_aug[h][:, kc, 0:64], in_=ps[:, bass.ts(h, 64)]
                        )

        exp_dve = os.environ.get("EXP_DVE", "1") == "1"

        def emit_exp(sa_t, sb_t, pts_t, ptv_t, sc_cols, dve_cols):
            """exp of a score group split across two PSUM tiles (sa for
            ScalarE true exp -> pts bf16, sb for the DVE bit-trick -> ptv
            int16 holding bf16 bit patterns). Separate PSUM tiles matter:
            reads of one PSUM tile are serialized by the dep tracker, so a
            shared tile would chain DVE behind ScalarE."""
            nc.scalar.activation(
                out=pts_t[:, 0:sc_cols], in_=sa_t[:, 0:sc_cols],
                func=mybir.ActivationFunctionType.Exp,
            )
            if dve_cols <= 0:
                return
            if exp_dve:
                nc.vector.tensor_scalar(
                    out=ptv_t[:, 0:dve_cols],
                    in0=sb_t[:, 0:dve_cols],
                    scalar1=float(EXPA), scalar2=float(EXPB),
                    op0=mybir.AluOpType.mult, op1=mybir.AluOpType.add,
                )
            else:
                nc.scalar.activation(
                    out=ptv_t[:, 0:dve_cols].bitcast(md),
                    in_=sb_t[:, 0:dve_cols],
                    func=mybir.ActivationFunctionType.Exp,
                )

        def p_slice(pts_t, ptv_t, half, c):
            """MM2 rhs for the 512-col chunk at interleaved column c."""
            if c < half:
                return pts_t[:, c:c + 512]
            return ptv_t[:, c - half:c - half + 512].bitcast(md)

        def normalize(o_t, dst64, obp, rsbp, dscr_p):
            """Drain o_t [65,512] PSUM, divide rows 0:64 by row 64, write
            the normalized bf16 result to dst64 (a [64,512]-shaped AP on
            partitions 0:64). Returns nothing; all off the PE."""
            ob = obp.tile([128, 512], F32, tag="ob")
            nc.vector.tensor_copy(out=ob[0:65, :], in_=o_t[0:65, :])
            scr = dscr_p.tile([512], F32, tag="scr")
            nc.sync.dma_start(out=scr, in_=ob[64:65, :])
            r4 = rsbp.tile([128, 4], F32, tag="r4")
            nc.sync.dma_start(out=r4, in_=scr.rearrange("(p f) -> p f", p=128))
            r4i = rsbp.tile([128, 4], F32, tag="r4i")
            nc.vector.reciprocal(out=r4i, in_=r4)
            scr2 = dscr_p.tile([512], F32, tag="scr2")
            nc.sync.dma_start(out=scr2, in_=r4i)
            bcs = rsbp.tile([64, 512], F32, tag="bcs")
            scr_b = bass.AP(tensor=scr2.tensor, offset=scr2.offset,
                            ap=[[0, 64]] + list(scr2.ap))
            nc.sync.dma_start(out=bcs, in_=scr_b)
            nc.vector.tensor_mul(dst64, ob[0:64, :], bcs)

        # ---- Loop 1: heads h0+h1 attention (row-tiled MM1 pairs) ----
        # S layout interleaves heads by slot: slot t writes h0 -> col 2t*512,
        # h1 -> (2t+1)*512, so ScalarE's half of the group is complete early.
        with (
            tc.tile_pool(name="s01a", bufs=1, space="PSUM") as s01ap,
            tc.tile_pool(name="s01b", bufs=1, space="PSUM") as s01bp,
            tc.tile_pool(name="o01", bufs=2, space="PSUM") as o01p,
            tc.tile_pool(name="pts1", bufs=2) as pts1p,
            tc.tile_pool(name="ptv1", bufs=2) as ptv1p,
            tc.tile_pool(name="ob1", bufs=2) as obp,
            tc.tile_pool(name="rs1", bufs=4) as rsbp,
            tc.tile_pool(name="ot1t", bufs=2) as ot1tp,
            tc.tile_pool(name="dscr", bufs=6, space="DRAM") as dscr_p,
        ):
            prev = None      # deferred (pts, ptv, half, ks, gsize, o_t0, o_t1)
            pend_fin = None  # deferred normalize for (o_t0, o_t1, j)

            def emit_mm2_l1(u):
                pts_t, ptv_t, half, ks, gsize, o_t0, o_t1 = u
                for t in range(gsize):
                    kc = ks + t
                    for h, o_t in ((0, o_t0), (1, o_t1)):
                        nc.tensor.matmul(
                            o_t[0:65, :], v_aug[h][:, kc, :],
                            p_slice(pts_t, ptv_t, half, (2 * t + h) * 512),
                            start=(kc == 0), stop=(kc == nk - 1),
                        )

            def finish_l1(u):
                o_t0, o_t1, j = u
                normalize(o_t0, ot01s[j][0:64, :], obp, rsbp, dscr_p)
                ot1 = ot1tp.tile([64, 512], md, tag="ot1")
                normalize(o_t1, ot1, obp, rsbp, dscr_p)
                nc.sync.dma_start(out=ot01s[j][64:128, :], in_=ot1)

            for j in range(nj):
                jsl = bass.ts(j, 512)
                o_t0 = o01p.tile([128, 512], F32, tag="o", name="ot0")
                o_t1 = o01p.tile([128, 512], F32, tag="o", name="ot1")
                ks = 0
                for gi, gsize in enumerate(groups1):
                    s_a = s01ap.tile([128, 2048], F32, tag="sa")
                    s_b = s01bp.tile([128, 1024], F32, tag="sb")
                    # ScalarE is the faster PSUM-exp path: give it ~2/3
                    sc_cols = min(2 * gsize - 1, 4) * 512
                    dve_cols = 2 * gsize * 512 - sc_cols
                    for t in range(gsize):
                        kc = ks + t
                        for h in range(2):
                            c = (2 * t + h) * 512
                            dst = (s_a[:, c:c + 512] if c < sc_cols
                                   else s_b[:, c - sc_cols:c - sc_cols + 512])
                            # h0 on array rows 0:63, h1 on 64:127 — concurrent
                            nc.tensor.matmul(
                                dst,
                                kt01[h * 64:(h + 1) * 64, bass.ts(kc, 128)],
                                qt01[h * 64:(h + 1) * 64, jsl],
                                start=True, stop=True,
                            )
                    if prev is not None:
                        emit_mm2_l1(prev)
                        prev = None
                    pts_t = pts1p.tile([128, 2048], md, tag="pts")
                    ptv_t = ptv1p.tile([128, 1024], mybir.dt.int16, tag="ptv")
                    emit_exp(s_a, s_b, pts_t, ptv_t, sc_cols, dve_cols)
                    if pend_fin is not None:
                        finish_l1(pend_fin)
                        pend_fin = None
                    prev = (pts_t, ptv_t, sc_cols, ks, gsize, o_t0, o_t1)
                    ks += gsize
                # the last group's MM2 lands in next j's first block; o_t is
                # complete after it, so queue the normalize then.
                pend_fin = (o_t0, o_t1, j)
            if prev is not None:
                emit_mm2_l1(prev)
                prev = None
            if pend_fin is not None:
                finish_l1(pend_fin)
                pend_fin = None

        # ---- Loop 2: head h2 (even/odd chunk pairing) + outproj ----
        with (
            tc.tile_pool(name="s2a", bufs=1, space="PSUM") as s2ap,
            tc.tile_pool(name="s2b", bufs=1, space="PSUM") as s2bp,
            tc.tile_pool(name="o2", bufs=2, space="PSUM") as o2p,
            tc.tile_pool(name="yps", bufs=2, space="PSUM") as ypsp,
            tc.tile_pool(name="pts2", bufs=2) as pts2p,
            tc.tile_pool(name="ptv2", bufs=2) as ptv2p,
            tc.tile_pool(name="ob2", bufs=2) as obp2,
            tc.tile_pool(name="rs2", bufs=4) as rsbp2,
            tc.tile_pool(name="ytp", bufs=4) as ytp,
            tc.tile_pool(name="dscr2", bufs=4, space="DRAM") as dscr2_p,
        ):
            prev2 = None      # deferred (pts, ptv, g, o_t2)
            pend_fin2 = None  # deferred normalize (o_t2, j)
            op_q = []         # outproj cc-pair queue [(j, cc, cc2), ...]

            def emit_mm2_l2(u):
                pts_t, ptv_t, g, o_t2 = u
                for t in range(4):
                    kc = 4 * g + t
                    nc.tensor.matmul(
                        o_t2[0:65, :], v_aug[2][:, kc, :],
                        p_slice(pts_t, ptv_t, 1536, t * 512),
                        start=(kc == 0), stop=(kc == nk - 1),
                    )

            def finish_l2(u):
                o_t2, j = u
                normalize(o_t2, ot2ds[j][0:64, :], obp2, rsbp2, dscr2_p)
                nc.sync.dma_start(out=ot2ds[j][64:128, :], in_=ot2ds[j][0:64, :])

            def emit_op_pair(j, cc_a, cc_b):
                pjsl = bass.ts(j, 512)
                yps_a = ypsp.tile([128, 512], F32, tag="yps", name="ypsa")
                yps_b = ypsp.tile([128, 512], F32, tag="yps", name="ypsb")
                nc.tensor.matmul(yps_a, wp01_sb[:, bass.ts(cc_a, 128)],
                                 ot01s[j], start=True, stop=False)
                nc.tensor.matmul(yps_b, wp01_sb[:, bass.ts(cc_b, 128)],
                                 ot01s[j], start=True, stop=False)
                # the two K=64 wp2 parts run concurrently (rows 0:63 / 64:127)
                nc.tensor.matmul(yps_a, wp2d_sb[0:64, bass.ts(cc_a, 128)],
                                 ot2ds[j][0:64, :], start=False, stop=True)
                nc.tensor.matmul(yps_b, wp2d_sb[64:128, bass.ts(cc_b, 128)],
                                 ot2ds[j][64:128, :], start=False, stop=True)
                # alternate drain engine and DMA queue so the stores overlap
                for yps, cc, qeng in ((yps_a, cc_a, 0), (yps_b, cc_b, 1)):
                    yst = ytp.tile([128, 512], F32, tag="yt")
                    if qeng == 0:
                        nc.vector.tensor_copy(out=yst, in_=yps)
                        nc.sync.dma_start(out=yt_d[bass.ts(cc, 128), pjsl], in_=yst)
                    else:
                        nc.scalar.copy(out=yst, in_=yps)
                        nc.gpsimd.dma_start(out=yt_d[bass.ts(cc, 128), pjsl], in_=yst)

            for j in range(nj):
                jsl = bass.ts(j, 512)
                o_t2 = o2p.tile([128, 512], F32, tag="o2")
                if j > 0:
                    op_q.extend([(j - 1, 0, 1), (j - 1, 2, 3), (j - 1, 4, 5)])
                for g in range(ng2):
                    s_a = s2ap.tile([128, 1536], F32, tag="sa2")
                    s_b = s2bp.tile([128, 512], F32, tag="sb2")
                    for t in range(4):
                        # chunk 4g+t at interleaved col t*512; t<3 -> s_a
                        kc = 4 * g + t
                        dst = (s_a[:, t * 512:(t + 1) * 512] if t < 3
                               else s_b[:, 0:512])
                        hb = (t % 2) * 64
                        nc.tensor.matmul(
                            dst,
                            kt2d[hb:hb + 64, bass.ts(kc, 128)],
                            qt2d[hb:hb + 64, jsl],
                            start=True, stop=True,
                        )
                    if prev2 is not None:
                        emit_mm2_l2(prev2)
                        prev2 = None
                    # spread the deferred output projection over late groups
                    if op_q and g >= max(1, ng2 - 5) and g % 2 == 1:
                        emit_op_pair(*op_q.pop(0))
                    pts_t = pts2p.tile([128, 1536], md, tag="pts2")
                    ptv_t = ptv2p.tile([128, 512], mybir.dt.int16, tag="ptv2")
                    emit_exp(s_a, s_b, pts_t, ptv_t, 1536, 512)
                    if pend_fin2 is not None:
                        finish_l2(pend_fin2)
                        pend_fin2 = None
                    prev2 = (pts_t, ptv_t, g, o_t2)
                while op_q:
                    emit_op_pair(*op_q.pop(0))
                pend_fin2 = (o_t2, j)
            if prev2 is not None:
                emit_mm2_l2(prev2)
                prev2 = None
            if pend_fin2 is not None:
                finish_l2(pend_fin2)
                pend_fin2 = None
            for pair in ((nj - 1, 0, 1), (nj - 1, 2, 3), (nj - 1, 4, 5)):
                emit_op_pair(*pair)

        persist.release()
        consts.release()

    nc.compile()
    return nc


def make_core_inputs(x_b, w_qkv, w_proj, hbase, fast_mm=True):
    """Per-core input arrays for heads [hbase, hbase+3) of batch x_b."""
    C = DIM
    wq = [w_qkv[(hbase + h) * 64:(hbase + h + 1) * 64, :] * SCALE for h in range(3)]
    wk = [w_qkv[C + (hbase + h) * 64:C + (hbase + h + 1) * 64, :] for h in range(3)]
    wv = [w_qkv[2 * C + (hbase + h) * 64:2 * C + (hbase + h + 1) * 64, :] for h in range(3)]

    wqk = np.zeros((C, 384), np.float32)
    wqk[:, 0:64] = wq[0].T
    wqk[:, 64:128] = wq[1].T
    wqk[:, 128:192] = wk[0].T
    wqk[:, 192:256] = wk[1].T
    wqk[:, 256:320] = wq[2].T
    wqk[:, 320:384] = wk[2].T

    wv_p = np.zeros((C, 192), np.float32)
    for h in range(3):
        wv_p[:, h * 64:(h + 1) * 64] = wv[h].T

    wp = np.zeros((192, C), np.float32)
    for h in range(3):
        wp[h * 64:(h + 1) * 64, :] = w_proj[:, (hbase + h) * 64:(hbase + h + 1) * 64].T

    dt = ml_dtypes.bfloat16 if fast_mm else np.float32
    return {
        "xt": np.ascontiguousarray(x_b.T).astype(dt),
        "wqk": wqk.astype(dt),
        "wv": wv_p.astype(dt),
        "wp": wp.astype(dt),
    }


_NC_CACHE = {}


def get_nc(n=N_FULL, fast_mm=True, pace=True):
    key = (n, fast_mm, pace)
    if key not in _NC_CACHE:
        _NC_CACHE[key] = build_nc(n, fast_mm, pace)
    return _NC_CACHE[key]


def kernel(x, w_qkv, w_proj, b_proj, _trace=False):
    x = np.asarray(x, np.float32)
    w_qkv = np.asarray(w_qkv, np.float32)
    w_proj = np.asarray(w_proj, np.float32)
    b_proj = np.asarray(b_proj, np.float32)

    nc = get_nc(N_FULL, True)
    in_maps = []
    for c in range(N_CORES):
        b = c // 4
        hbase = (c % 4) * HEADS_PER_CORE
        in_maps.append(make_core_inputs(x[b], w_qkv, w_proj, hbase))

    res = run_bass_kernel_spmd(nc, in_maps, core_ids=list(range(N_CORES)),
                               trace=_trace)
    y = np.empty((B, N_FULL, DIM), np.float32)
    for b in range(B):
        acc = res.results[4 * b]["yt"].astype(np.float32)
        for c in range(4 * b + 1, 4 * b + 4):
            acc = acc + res.results[c]["yt"]
        y[b] = acc.T + b_proj[None, :]
    if _trace:
        return y, res
    return y
